# revision 1
# baseline (speedup 1.0000x reference)
"""Trainium2 Bass kernel for a 2-layer GAT (nn_GAT_83382495084588).

Distribution (8 NeuronCores, one chip, pure SPMD — one program, per-core data):
  - dst-node sharding: nodes are lex-sorted by (a, b) = per-node counts of
    src's in table-half A/B, chunked into 128-node tiles, snake-dealt to cores
    so every core has the same per-round slot schedule (DA[r], DB[r]).
  - Phase 0 (replicated): every core computes the full Z1 table
    [feat1.fp16 | el1.f32 | er1.f32] (h @ [W1 | W1@al1_bd | W1@ar1_bd], fp32r
    matmul) into its own DRAM — cheaper than a 51MB AllGather of feat1.
  - Layer-1 edge phase: per (round, half) one dma_gather (int16 idx) fetches
    768B packed rows for all slots incl. a leading self-row (supplies er of
    the dst node itself; the opposite half's self-row points at a zero row).
    Segment softmax over padded slots (additive -1e30 mask, no max-subtract;
    |e| <= ~4 verified), then per-slot msg = feat*alpha on DVE (out fp32r)
    accumulated with identity-matmuls into PSUM.
  - Inter-layer: each core scatters its compact z2 rows [feat2.fp16|el2|er2]
    (136B) into an A|B-ordered shard via per-round indirect row scatters, two
    AllGathers (A and B halves, ~3.4MB each) exchange them, then a DRAM->DRAM
    repack pads rows to 256B for gathering.
  - Layer-2 edge phase mirrors layer 1 (1 head, 64 dims, er2 is core-local).

kernel(**inputs) takes the full unsharded inputs and returns the full
(50000, 64) output; host numpy does only sharding/index prep + unshuffle.
"""

import os
import sys
from dataclasses import dataclass, field

import numpy as np

for _p in ("/opt/trn_rl_repo", "/root/.axon_site/_ro/trn_rl_repo"):
    if os.path.isdir(_p) and _p not in sys.path:
        sys.path.append(_p)

import concourse.bacc as bacc
import concourse.bass as bass
import concourse.mybir as mybir
import concourse.tile as tile
from concourse.bass import IndirectOffsetOnAxis
from concourse.bass_utils import run_bass_kernel_spmd

F32 = mybir.dt.float32
F32R = mybir.dt.float32r
F16 = mybir.dt.float16
I32 = mybir.dt.int32
I16 = mybir.dt.int16
AF = mybir.ActivationFunctionType
OP = mybir.AluOpType

P = 128
NCC = 8
PHASES = int(os.environ.get("GAT_PHASES", "4"))
L1SUB = int(os.environ.get("GAT_L1SUB", "0"))
NEG_SLOPE = 0.2
F32R_INPUTS = {"htiles", "W1ext", "W2ext", "identr"}
I16_INPUTS = {"gidx1", "gidx2"}


@dataclass
class Prob:
    N: int
    IN_DIM: int
    H1: int
    HID: int
    OUT_DIM: int
    rounds: int = 0
    DA: list = field(default_factory=list)
    DB: list = field(default_factory=list)
    # derived
    C1: int = 0        # H1*HID
    Z1W: int = 0       # L1 table width in f32 words (768B -> 192)
    Z2W: int = 0       # L2 gather-table width in f32 words (256B -> 64)
    Z2C: int = 0       # compact z2 row width in f32 words (34)
    NT: int = 0        # rounds * P (nodes per core)
    HALF: int = 0      # rows per table half
    SD: int = 0        # sum(DA) + sum(DB)
    l2_groups: list = field(default_factory=list)  # list of round-ranges

    def finish(self):
        self.C1 = self.H1 * self.HID
        self.Z1W = self.C1 // 2 + 2 * self.H1 + 48   # fp16-packed feat + el + er + pad
        self.Z2W = 64
        self.Z2C = 64
        self.NT = self.rounds * P
        self.HALF = NCC * self.NT // 2
        self.SD = int(sum(self.DA) + sum(self.DB))
        return self


def wrap16(flat_idx):
    """dma_gather idx layout: flat i -> [i%16, i//16], replicated to 128 rows."""
    n = len(flat_idx)
    S = max(1, (n + 15) // 16)
    t = np.zeros((16, S), np.int16)
    ii = np.arange(n)
    t[ii % 16, ii // 16] = flat_idx
    return np.tile(t, (8, 1))


def prep_all(inputs, pr: Prob):
    rng = np.random.default_rng(12345)
    src = np.asarray(inputs["src"]).astype(np.int64)
    dst = np.asarray(inputs["dst"]).astype(np.int64)
    h = np.asarray(inputs["h"], dtype=np.float32)
    W1 = np.asarray(inputs["W1"], dtype=np.float32)
    al1 = np.asarray(inputs["al1"], dtype=np.float32)
    ar1 = np.asarray(inputs["ar1"], dtype=np.float32)
    b1 = np.asarray(inputs["b1"], dtype=np.float32)
    W2 = np.asarray(inputs["W2"], dtype=np.float32)
    al2 = np.asarray(inputs["al2"], dtype=np.float32)
    ar2 = np.asarray(inputs["ar2"], dtype=np.float32)
    b2 = np.asarray(inputs["b2"], dtype=np.float32)
    N = pr.N

    deg = np.bincount(dst, minlength=N)
    NT_G = ((N + NCC * P - 1) // (NCC * P)) * (NCC * P)   # padded global nodes
    rounds = NT_G // (NCC * P)
    n_dummy = NT_G - N

    # --- initial random half designation over real nodes ---
    desA = np.zeros(N, bool)
    desA[rng.permutation(N)[:N // 2]] = True

    # --- (a, b) counts and lex sort ---
    cA = np.zeros(N, np.int64)
    np.add.at(cA, dst, desA[src].astype(np.int64))
    cB = deg - cA
    order = np.lexsort((cB, cA))
    gnodes = np.concatenate([order, np.full(n_dummy, -1, np.int64)])

    # --- snake deal to cores ---
    assign = np.zeros((NCC, rounds), np.int64)
    for r in range(rounds):
        for c in range(NCC):
            assign[c, r] = r * NCC + (c if r % 2 == 0 else NCC - 1 - c)
    core_nodes = np.zeros((NCC, rounds * P), np.int64)
    for c in range(NCC):
        for r in range(rounds):
            t = assign[c, r]
            core_nodes[c, r * P:(r + 1) * P] = gnodes[t * P:(t + 1) * P]

    # --- rebalance: each core must own exactly NT/2 A-designated rows.
    # dummies are free to designate; then flip real nodes if still uneven.
    halfNT = rounds * P // 2
    desA_d = {}   # designation of dummy slots per (core, pos)
    for c in range(NCC):
        nodes = core_nodes[c]
        real = nodes[nodes >= 0]
        nA = int(desA[real].sum())
        dummies = np.where(nodes < 0)[0]
        need = halfNT - nA
        take = max(0, min(len(dummies), need))
        for j, posi in enumerate(dummies):
            desA_d[(c, int(posi))] = j < take
        need -= take
        if need > 0:      # designate more real B -> A
            bsel = real[~desA[real]]
            desA[bsel[:need]] = True
        elif need < 0:    # demote some real A -> B
            asel = real[desA[real]]
            desA[asel[:(-need)]] = False

    # recompute (a, b) under final designation (tiles unchanged)
    cA = np.zeros(N, np.int64)
    np.add.at(cA, dst, desA[src].astype(np.int64))
    cB = deg - cA
    DA = np.zeros(rounds, np.int64)
    DB = np.zeros(rounds, np.int64)
    for r in range(rounds):
        sel = gnodes[r * NCC * P:(r + 1) * NCC * P]
        realr = sel[sel >= 0]
        DA[r] = max(1, int(cA[realr].max()) if len(realr) else 1)
        DB[r] = max(1, int(cB[realr].max()) if len(realr) else 1)

    pr.rounds = rounds
    pr.DA = [int(x) for x in DA]
    pr.DB = [int(x) for x in DB]
    pr.finish()

    # --- L1 table order: all A rows then all B rows (core/position major) ---
    # row content for dummies is zero (h row = 0).
    pos1 = np.full(N, -1, np.int64)          # node -> L1 table row
    h_table = np.zeros((NCC * pr.NT, pr.IN_DIM), np.float32)
    ra, rb = 0, pr.HALF
    # also L2 shard ranks: node -> (core, rank within core's A or B shard part)
    shard_rank = np.full(NCC * pr.NT, -1, np.int64)   # per (c, pos)
    pos2 = np.full(N, -1, np.int64)          # node -> L2 table row (A:0..HALF)
    for c in range(NCC):
        ca = cb = 0
        for posi in range(pr.NT):
            n = core_nodes[c, posi]
            if n >= 0:
                isa = bool(desA[n])
            else:
                isa = desA_d.get((c, posi), False)
            if isa:
                if n >= 0:
                    pos1[n] = ra
                    h_table[ra] = h[n]
                    pos2[n] = c * halfNT + ca
                shard_rank[c * pr.NT + posi] = ca
                ra += 1
                ca += 1
            else:
                if n >= 0:
                    pos1[n] = rb
                    h_table[rb] = h[n]
                    pos2[n] = NCC * halfNT + c * halfNT + cb
                shard_rank[c * pr.NT + posi] = halfNT + cb
                rb += 1
                cb += 1
    assert ra == pr.HALF and rb == 2 * pr.HALF

    # --- CSR by dst, split per node into A-edges / B-edges ---
    sort = np.argsort(dst, kind="stable")
    s_src = src[sort]
    starts = np.zeros(N + 1, np.int64)
    np.cumsum(deg, out=starts[1:])

    offs = np.zeros(rounds + 1, np.int64)
    np.cumsum(np.array(pr.DA) + np.array(pr.DB), out=offs[1:])

    per_core = []
    for c in range(NCC):
        nodes = core_nodes[c]
        mask = np.full((P, pr.SD), np.float32(-1e30), np.float32)
        gi1 = []   # int16 idx stream for L1 gathers (per round: A then B unit)
        gi2 = []   # for L2
        scat = np.zeros((P, rounds), np.int32)
        selfA = np.zeros((P, rounds), np.int32)
        for r in range(rounds):
            da, db = pr.DA[r], pr.DB[r]
            off = offs[r]
            iA1 = np.zeros((1 + da, P), np.int64)   # [slot, partition]
            iB1 = np.zeros((1 + db, P), np.int64)
            iA2 = np.zeros((da, P), np.int64)
            iB2 = np.zeros((db, P), np.int64)
            for p in range(P):
                n = nodes[r * P + p]
                scat[p, r] = shard_rank[c * pr.NT + r * P + p]
                if n < 0:
                    mask[p, off] = 0.0   # one live A-slot (row 0) keeps denom > 0
                    continue
                if desA[n]:
                    iA1[0, p] = pos1[n]
                    selfA[p, r] = 1
                else:
                    iB1[0, p] = pos1[n] - pr.HALF
                srcs = s_src[starts[n]:starts[n + 1]]
                sa = srcs[desA[srcs]]
                sb = srcs[~desA[srcs]]
                iA1[1:1 + len(sa), p] = pos1[sa]
                iB1[1:1 + len(sb), p] = pos1[sb] - pr.HALF
                iA2[:len(sa), p] = pos2[sa]
                iB2[:len(sb), p] = pos2[sb] - NCC * halfNT
                mask[p, off:off + len(sa)] = 0.0
                mask[p, off + da:off + da + len(sb)] = 0.0
            gi1.append(iA1.reshape(-1))
            gi1.append(iB1.reshape(-1))
            gi2.append(iA2.reshape(-1))
            gi2.append(iB2.reshape(-1))
        gidx1 = wrap16(np.concatenate(gi1))
        gidx2 = wrap16(np.concatenate(gi2))
        per_core.append(dict(gidx1=gidx1.astype(np.int16),
                             gidx2=gidx2.astype(np.int16),
                             mask=mask, scat=scat, selfA=selfA))

    # --- htiles for phase 0 (L1-table order, transposed, tiled) ---
    T0 = (NCC * pr.NT) // P
    ht = h_table.reshape(T0, P, 2, P).transpose(3, 0, 2, 1).reshape(P, T0 * 2, P)
    htiles = np.ascontiguousarray(ht)

    # --- extended weights (f64 host precompute) ---
    H1n, HID = pr.H1, pr.HID
    al_bd = np.zeros((pr.C1, H1n), np.float64)
    ar_bd = np.zeros((pr.C1, H1n), np.float64)
    for hh in range(H1n):
        al_bd[hh * HID:(hh + 1) * HID, hh] = al1[hh].astype(np.float64)
        ar_bd[hh * HID:(hh + 1) * HID, hh] = ar1[hh].astype(np.float64)
    W1f = W1.astype(np.float64)
    W1ext = np.concatenate([W1, (W1f @ al_bd).astype(np.float32),
                            (W1f @ ar_bd).astype(np.float32)], axis=1)
    W2f = W2.astype(np.float64)
    W2ext = np.concatenate(
        [W2, (W2f @ al2.astype(np.float64).reshape(-1, 1)).astype(np.float32),
         (W2f @ ar2.astype(np.float64).reshape(-1, 1)).astype(np.float32)], axis=1)

    shared = dict(
        htiles=htiles,
        W1ext=np.ascontiguousarray(W1ext),
        W2ext=np.ascontiguousarray(W2ext),
        identr=np.eye(P, dtype=np.float32),
        identf=np.eye(P, dtype=np.float32),
        b1rep=np.broadcast_to(b1, (P, pr.C1)).copy(),
        b2rep=np.broadcast_to(b2, (P, pr.OUT_DIM)).copy(),
    )
    in_maps = []
    for c in range(NCC):
        m = dict(shared)
        m.update(per_core[c])
        in_maps.append(m)
    sched = dict(core_nodes=core_nodes, rounds=rounds)
    return sched, in_maps


def build_kernel_fn(pr: Prob):
    rounds, DA, DB = pr.rounds, pr.DA, pr.DB
    C1, Z1W, Z2W, Z2C = pr.C1, pr.Z1W, pr.Z2W, pr.Z2C
    H1, HID, OUT = pr.H1, pr.HID, pr.OUT_DIM
    T0 = (NCC * pr.NT) // P
    HALF = pr.HALF
    halfNT = pr.NT // 2
    F16C = C1 // 2            # feat fp16 packed into f32 words
    EL0 = F16C                # el word offset in Z1 row
    ER0 = F16C + H1
    offs = np.zeros(rounds + 1, np.int64)
    np.cumsum(np.array(DA) + np.array(DB), out=offs[1:])
    # idx stream offsets (in int16-wrapped columns: 1 col = 16 idx)
    gi1_off, gi2_off = [0], [0]
    for r in range(rounds):
        gi1_off.append(gi1_off[-1] + ((1 + DA[r]) + (1 + DB[r])) * 8)
        gi2_off.append(gi2_off[-1] + (DA[r] + DB[r]) * 8)

    def kern(tc: tile.TileContext, outs, ins):
        nc = tc.nc

        Z1 = nc.dram_tensor("Z1d", [2 * HALF, Z1W], F32)
        z2shard = nc.dram_tensor("z2shardd", [pr.NT, Z2C], F32)
        Z2 = nc.dram_tensor("Z2d", [2 * HALF, Z2W], F32, addr_space="Shared")

        with (
            tc.tile_pool(name="const", bufs=1) as cpool,
            tc.tile_pool(name="big", bufs=1) as big,
        ):
            # ---- constants ----
            w1e = cpool.tile([P, 2, C1 + 2 * H1], F32R)
            for c in range(2):
                nc.sync.dma_start(w1e[:, c, :], ins["W1ext"][c * P:(c + 1) * P, :])
            w2e = cpool.tile([P, 2, OUT + 2], F32R)
            for c in range(2):
                nc.sync.dma_start(w2e[:, c, :], ins["W2ext"][c * P:(c + 1) * P, :])
            ident = cpool.tile([P, P], F32R)
            nc.sync.dma_start(ident[:], ins["identr"][:, :])
            identf = cpool.tile([P, P], F32)
            nc.sync.dma_start(identf[:], ins["identf"][:, :])
            b1r = cpool.tile([P, C1], F32)
            nc.sync.dma_start(b1r[:], ins["b1rep"][:, :])
            b2r = cpool.tile([P, OUT], F32)
            nc.sync.dma_start(b2r[:], ins["b2rep"][:, :])
            gidx1 = cpool.tile([P, gi1_off[-1]], I16)
            nc.sync.dma_start(gidx1[:], ins["gidx1"][:, :])
            gidx2 = cpool.tile([P, gi2_off[-1]], I16)
            nc.sync.dma_start(gidx2[:], ins["gidx2"][:, :])
            maskt = cpool.tile([P, pr.SD], F32)
            nc.sync.dma_start(maskt[:], ins["mask"][:, :])
            scatt = cpool.tile([P, rounds], I32)
            nc.sync.dma_start(scatt[:], ins["scat"][:, :])
            selfat = cpool.tile([P, rounds], I32)
            nc.sync.dma_start(selfat[:], ins["selfA"][:, :])

            h1all = big.tile([P, rounds, C1], F32)
            z2all = big.tile([P, rounds, Z2C], F32)
            nc.vector.memset(z2all[:, :, OUT // 2 + 2:Z2C], 0.0)

            if PHASES < 1:
                return
            # ---- phase 0: Z1 = h @ [W1|W1al|W1ar] (replicated; fp16 feat) ----
            with (
                tc.tile_pool(name="p0h", bufs=4) as p0h,
                tc.tile_pool(name="p0ps", bufs=4, space="PSUM") as p0ps,
                tc.tile_pool(name="p0z", bufs=4) as p0z,
            ):
                for t in range(T0):
                    ht = p0h.tile([P, 2, P], F32R)
                    nc.sync.dma_start(ht[:], ins["htiles"][:, 2 * t:2 * t + 2, :])
                    zps = p0ps.tile([P, C1 + 2 * H1], F32)
                    nc.tensor.matmul(zps[:], lhsT=ht[:, 0, :],
                                     rhs=w1e[:, 0, :], start=True, stop=False)
                    nc.tensor.matmul(zps[:], lhsT=ht[:, 1, :],
                                     rhs=w1e[:, 1, :], start=False, stop=True)
                    zsb = p0z.tile([P, Z1W], F32)
                    nc.vector.tensor_copy(
                        zsb[:, 0:F16C].bitcast(F16), zps[:, 0:C1])
                    nc.vector.tensor_copy(
                        zsb[:, EL0:ER0 + H1], zps[:, C1:C1 + 2 * H1])
                    nc.vector.memset(zsb[:, ER0 + H1:Z1W], 0.0)
                    nc.sync.dma_start(Z1[t * P:(t + 1) * P, :], zsb[:])

            if PHASES < 2:
                return
            # ---- layer-1 edge phase ----
            with (
                tc.tile_pool(name="fg", bufs=2) as fgp,
                tc.tile_pool(name="al", bufs=3) as alp,
                tc.tile_pool(name="sm", bufs=4) as smp,
                tc.tile_pool(name="msg", bufs=4) as msgp,
                tc.tile_pool(name="l1ps", bufs=2, space="PSUM") as l1ps,
                tc.tile_pool(name="ep", bufs=3) as epp,
            ):
                for r in range(rounds):
                    da, db = DA[r], DB[r]
                    off = int(offs[r])
                    c0 = gi1_off[r]
                    cab = (1 + da) * 8
                    idxA = fgp.tile([P, (1 + da) * 8], I16, tag="idxA")
                    nc.sync.dma_start(idxA[:], ins["gidx1"][:, c0:c0 + cab])
                    idxB = fgp.tile([P, (1 + db) * 8], I16, tag="idxB")
                    nc.sync.dma_start(idxB[:],
                                      ins["gidx1"][:, c0 + cab:c0 + cab + (1 + db) * 8])
                    gA = fgp.tile([P, 1 + da, Z1W], F32, tag="gA")
                    nc.gpsimd.dma_gather(gA[:], Z1[0:HALF, :], idxA[:],
                                         (1 + da) * P, (1 + da) * P, Z1W,
                                         single_packet=False)
                    gB = fgp.tile([P, 1 + db, Z1W], F32, tag="gB")
                    nc.gpsimd.dma_gather(gB[:], Z1[HALF:2 * HALF, :], idxB[:],
                                         (1 + db) * P, (1 + db) * P, Z1W,
                                         single_packet=False)
                    if L1SUB == 1:
                        continue
                    # er = A-half self-row if node designated A else B-half's
                    er = alp.tile([P, H1], F32, tag="er")
                    nc.vector.tensor_copy(er[:], gB[:, 0, ER0:ER0 + H1])
                    nc.vector.copy_predicated(
                        er[:], selfat[:, r:r + 1].to_broadcast((P, H1)),
                        gA[:, 0, ER0:ER0 + H1])
                    # e = lrelu(el + er) + mask over the da+db edge slots
                    ew = alp.tile([P, da + db, H1], F32, tag="ew")
                    tmp = alp.tile([P, da + db, H1], F32, tag="tmp")
                    for (s0, dd_, gt) in ((0, da, gA), (da, db, gB)):
                        nc.vector.tensor_tensor(
                            out=ew[:, s0:s0 + dd_, :],
                            in0=gt[:, 1:1 + dd_, EL0:EL0 + H1],
                            in1=er[:, None, :].to_broadcast((P, dd_, H1)),
                            op=OP.add)
                    nc.vector.tensor_scalar_mul(tmp[:], ew[:], NEG_SLOPE)
                    nc.vector.tensor_tensor(out=ew[:], in0=ew[:], in1=tmp[:],
                                            op=OP.max)
                    nc.vector.tensor_tensor(
                        out=ew[:], in0=ew[:],
                        in1=maskt[:, off:off + da + db, None]
                            .to_broadcast((P, da + db, H1)),
                        op=OP.add)
                    nc.scalar.activation(out=ew[:], in_=ew[:], func=AF.Exp)
                    den = smp.tile([P, H1], F32, tag="den")
                    nc.vector.reduce_sum(
                        out=den[:], in_=ew[:].rearrange("p d h -> p h d"),
                        axis=mybir.AxisListType.X)
                    nc.vector.reciprocal(out=den[:], in_=den[:])
                    nc.vector.tensor_tensor(
                        out=ew[:], in0=ew[:],
                        in1=den[:, None, :].to_broadcast((P, da + db, H1)),
                        op=OP.mult)
                    # msg accumulation
                    ps = l1ps.tile([P, C1], F32)
                    for k in range(da + db):
                        gsl = gA[:, 1 + k, 0:F16C] if k < da \
                            else gB[:, 1 + k - da, 0:F16C]
                        mg = msgp.tile([P, C1], F32R, tag="mg")
                        nc.vector.tensor_tensor(
                            out=mg[:].rearrange("p (h d) -> p h d", h=H1),
                            in0=gsl.bitcast(F16)
                                .rearrange("p (h d) -> p h d", h=H1),
                            in1=ew[:, k, :, None].to_broadcast((P, H1, HID)),
                            op=OP.mult)
                        if L1SUB != 3:
                            nc.tensor.matmul(ps[:], lhsT=ident[:], rhs=mg[:],
                                             start=(k == 0), stop=(k == da + db - 1))
                    if L1SUB == 3:
                        continue
                    # epilogue: h1 = elu(psum + b1)
                    x = epp.tile([P, C1], F32, tag="x")
                    nc.vector.tensor_tensor(out=x[:], in0=ps[:], in1=b1r[:],
                                            op=OP.add)
                    mn = epp.tile([P, C1], F32, tag="mn")
                    nc.vector.tensor_scalar_min(mn[:], x[:], 0.0)
                    exn = epp.tile([P, C1], F32, tag="exn")
                    nc.scalar.activation(out=exn[:], in_=mn[:], func=AF.Exp)
                    nc.vector.tensor_scalar_max(x[:], x[:], 0.0)
                    nc.vector.tensor_tensor(out=h1all[:, r, :], in0=x[:],
                                            in1=exn[:], op=OP.add)
                    nc.vector.tensor_scalar_sub(h1all[:, r, :], h1all[:, r, :], 1.0)

            if PHASES < 3:
                return
            # ---- layer-2 matmul + shard scatter ----
            with (
                tc.tile_pool(name="tps", bufs=2, space="PSUM") as tpsp,
                tc.tile_pool(name="h1t", bufs=3) as h1tp,
                tc.tile_pool(name="z2ps", bufs=2, space="PSUM") as z2psp,
            ):
                for r in range(rounds):
                    tps = tpsp.tile([P, 2, P], F32)
                    for c in range(2):
                        nc.tensor.transpose(out=tps[:, c, :],
                                            in_=h1all[:, r, c * P:(c + 1) * P],
                                            identity=identf[:])
                    h1t = h1tp.tile([P, 2, P], F32R)
                    nc.vector.tensor_copy(h1t[:], tps[:])
                    z2ps = z2psp.tile([P, OUT + 2], F32)
                    for c in range(2):
                        nc.tensor.matmul(z2ps[:], lhsT=h1t[:, c, :],
                                         rhs=w2e[:, c, :],
                                         start=(c == 0), stop=(c == 1))
                    nc.vector.tensor_copy(
                        z2all[:, r, 0:OUT // 2].bitcast(F16), z2ps[:, 0:OUT])
                    nc.vector.tensor_copy(
                        z2all[:, r, OUT // 2:OUT // 2 + 2], z2ps[:, OUT:OUT + 2])
                    nc.gpsimd.indirect_dma_start(
                        out=z2shard[:, :], out_offset=IndirectOffsetOnAxis(
                            ap=scatt[:, r:r + 1], axis=0),
                        in_=z2all[:, r, :], in_offset=None)

            nc.gpsimd.collective_compute(
                "AllGather", OP.bypass, replica_groups=[list(range(NCC))],
                ins=[z2shard[0:halfNT, :]], outs=[Z2[0:HALF, :]])
            nc.gpsimd.collective_compute(
                "AllGather", OP.bypass, replica_groups=[list(range(NCC))],
                ins=[z2shard[halfNT:pr.NT, :]], outs=[Z2[HALF:2 * HALF, :]])

            if PHASES < 4:
                return
            # ---- layer-2 edge phase ----
            EL2 = OUT // 2
            ER2 = OUT // 2 + 1
            MAXDD = max(DA[i] + DB[i] for i in range(rounds))
            out_ap = outs["out"]
            with (
                tc.tile_pool(name="fg2", bufs=3) as fg2p,
                tc.tile_pool(name="al2", bufs=3) as al2p,
                tc.tile_pool(name="msg2", bufs=4) as msg2p,
                tc.tile_pool(name="l2ps", bufs=2, space="PSUM") as l2ps,
                tc.tile_pool(name="ep2", bufs=3) as ep2p,
            ):
                for r in range(rounds):
                    da, db = DA[r], DB[r]
                    off = int(offs[r])
                    dd = da + db
                    c0 = gi2_off[r]
                    idxA = fg2p.tile([P, da * 8], I16, tag="idx2A")
                    nc.sync.dma_start(idxA[:], ins["gidx2"][:, c0:c0 + da * 8])
                    idxB = fg2p.tile([P, db * 8], I16, tag="idx2B")
                    nc.sync.dma_start(idxB[:],
                                      ins["gidx2"][:, c0 + da * 8:c0 + dd * 8])
                    ggA = fg2p.tile([P, da, Z2W], F32, tag="fg2A")
                    nc.gpsimd.dma_gather(ggA[:], Z2[0:HALF, :], idxA[:],
                                         da * P, da * P, Z2W,
                                         single_packet=False)
                    ggB = fg2p.tile([P, db, Z2W], F32, tag="fg2B")
                    nc.gpsimd.dma_gather(ggB[:], Z2[HALF:2 * HALF, :], idxB[:],
                                         db * P, db * P, Z2W,
                                         single_packet=False)
                    ew = al2p.tile([P, MAXDD], F32, tag="ew2")
                    tmp = al2p.tile([P, MAXDD], F32, tag="tmp2")
                    for (s0, dd_, gt) in ((0, da, ggA), (da, db, ggB)):
                        nc.vector.tensor_tensor(
                            out=ew[:, s0:s0 + dd_], in0=gt[:, 0:dd_, EL2],
                            in1=z2all[:, r, ER2:ER2 + 1].to_broadcast((P, dd_)),
                            op=OP.add)
                    nc.vector.tensor_scalar_mul(tmp[:, 0:dd], ew[:, 0:dd],
                                                NEG_SLOPE)
                    nc.vector.tensor_tensor(out=ew[:, 0:dd], in0=ew[:, 0:dd],
                                            in1=tmp[:, 0:dd], op=OP.max)
                    nc.vector.tensor_tensor(out=ew[:, 0:dd], in0=ew[:, 0:dd],
                                            in1=maskt[:, off:off + dd],
                                            op=OP.add)
                    nc.scalar.activation(out=ew[:, 0:dd], in_=ew[:, 0:dd],
                                         func=AF.Exp)
                    den = al2p.tile([P, 1], F32, tag="den2")
                    nc.vector.reduce_sum(out=den[:], in_=ew[:, 0:dd],
                                         axis=mybir.AxisListType.X)
                    nc.vector.reciprocal(out=den[:], in_=den[:])
                    nc.vector.tensor_tensor(
                        out=ew[:, 0:dd], in0=ew[:, 0:dd],
                        in1=den[:].to_broadcast((P, dd)), op=OP.mult)
                    ps = l2ps.tile([P, OUT], F32)
                    for k in range(dd):
                        gsl = ggA[:, k, 0:OUT // 2] if k < da \
                            else ggB[:, k - da, 0:OUT // 2]
                        mg = msg2p.tile([P, OUT], F32R, tag="mg2")
                        nc.vector.tensor_tensor(
                            out=mg[:],
                            in0=gsl.bitcast(F16),
                            in1=ew[:, k, None].to_broadcast((P, OUT)),
                            op=OP.mult)
                        nc.tensor.matmul(ps[:], lhsT=ident[:], rhs=mg[:],
                                         start=(k == 0), stop=(k == dd - 1))
                    ot = ep2p.tile([P, OUT], F32, tag="ot")
                    nc.vector.tensor_tensor(out=ot[:], in0=ps[:], in1=b2r[:],
                                            op=OP.add)
                    nc.sync.dma_start(
                        out_ap[:].rearrange("(i p) c -> p i c", p=P)[:, r, :],
                        ot[:])

    return kern


def declare_io(nc, in_maps, pr: Prob):
    ins_ap = {}
    for k, v in in_maps[0].items():
        if k in F32R_INPUTS:
            dt = F32R
        else:
            dt = mybir.dt.from_np(v.dtype)
        ins_ap[k] = nc.dram_tensor(
            f"in_{k}", list(v.shape), dt, kind="ExternalInput").ap()
    outs_ap = {"out": nc.dram_tensor(
        "out", [pr.NT, pr.OUT_DIM], F32, kind="ExternalOutput").ap()}
    return ins_ap, outs_ap


def assemble_output(results, sched, pr: Prob):
    out = np.zeros((pr.N, pr.OUT_DIM), np.float32)
    for c in range(NCC):
        nodes = sched["core_nodes"][c]
        oc = results[c]["out"]
        valid = nodes >= 0
        out[nodes[valid]] = oc[valid]
    return out


def kernel(**inputs) -> np.ndarray:
    pr = Prob(N=50000, IN_DIM=256, H1=8, HID=32, OUT_DIM=64)
    sched, in_maps = prep_all(inputs, pr)

    nc = bacc.Bacc("TRN2", target_bir_lowering=False, debug=False,
                   num_devices=NCC)
    ins_ap, outs_ap = declare_io(nc, in_maps, pr)
    kern = build_kernel_fn(pr)
    with tile.TileContext(nc) as tc:
        kern(tc, outs_ap, ins_ap)
    nc.compile()

    maps = [{f"in_{k}": v for k, v in m.items()} for m in in_maps]
    res = run_bass_kernel_spmd(nc, maps, core_ids=list(range(NCC)))
    return assemble_output(res.results, sched, pr)


def kernel_timed(inputs):
    import time
    pr = Prob(N=50000, IN_DIM=256, H1=8, HID=32, OUT_DIM=64)
    t0 = time.perf_counter()
    sched, in_maps = prep_all(inputs, pr)
    t1 = time.perf_counter()
    nc = bacc.Bacc("TRN2", target_bir_lowering=False, debug=False,
                   num_devices=NCC)
    ins_ap, outs_ap = declare_io(nc, in_maps, pr)
    kern = build_kernel_fn(pr)
    with tile.TileContext(nc) as tc:
        kern(tc, outs_ap, ins_ap)
    nc.compile()
    t2 = time.perf_counter()
    maps = [{f"in_{k}": v for k, v in m.items()} for m in in_maps]
    res = run_bass_kernel_spmd(nc, maps, core_ids=list(range(NCC)))
    t3 = time.perf_counter()
    res = run_bass_kernel_spmd(nc, maps, core_ids=list(range(NCC)))
    t4 = time.perf_counter()
    print(f"prep {t1-t0:.1f}s compile {t2-t1:.1f}s run1 {t3-t2:.2f}s "
          f"run2 {t4-t3:.2f}s (exec+IO)")
    return assemble_output(res.results, sched, pr)


if __name__ == "__main__":
    import pickle
    with open("/tmp/inputs.pkl", "rb") as f:
        inputs = pickle.load(f)
    out = kernel_timed(inputs)
    exp = np.load("/tmp/expected_np.npy")
    rel = np.linalg.norm(out - exp) / np.linalg.norm(exp)
    print("Relative error:", rel)



# revision 55
# speedup vs baseline: 6.6505x; 6.6505x over previous
"""Trainium2 Bass kernel for a 2-layer GAT (nn_GAT_83382495084588).

Distribution (8 NeuronCores, pure SPMD — one program, per-core data):
  - dst-node sharding with a free A/B src-designation (greedy-balanced per
    dst) splitting the feature table into two int16-addressable halves;
    nodes lex-sorted by (a, b) counts per designation pool, dealt so every
    core/round tile holds 64 A-rows (partitions 0:63) and 64 B-rows
    (64:127) and all cores share the per-round slot schedule DA[r]/DB[r].
  - Phase 0 (sharded): each core computes z rows only for its own NT nodes
    (h @ [W1 | W1@al_bd | W1@ar_bd], fp16 matmul) writing a compact
    [NT, 768B] shard (feat fp16 512B + el f32 32B + pad); own-node
    el/er/feat stay in SBUF. Two AllGathers per layer exchange the
    A/B-half shards.
  - Edge phases: per round two dma_gathers (wrap16 int16 idx, 768B rows L1
    / 256B rows L2) fetch src feat+el; e = lrelu(el+er) + additive fp16
    mask for padded slots, exp on the scalar engine; self-edges never
    gathered (local feat/el/er + multiplicity m). Layer-2 el is recomputed
    on-chip (feat2 . al2). Messages fp16, accumulated with fp16
    identity-matmuls into PSUM, normalized once by 1/den after
    aggregation.
  - Layer-2 matmul (h1 transpose + W2ext) is fused into the layer-1 round
    loop; shard writes are contiguous (no scatters).

kernel(**inputs) takes the full unsharded inputs and returns the full
(50000, 64) float32 output; host numpy does sharding/index prep + unshuffle.
"""

import os
import sys
from dataclasses import dataclass, field

import numpy as np

for _p in ("/opt/trn_rl_repo", "/root/.axon_site/_ro/trn_rl_repo"):
    if os.path.isdir(_p) and _p not in sys.path:
        sys.path.append(_p)

import concourse.bacc as bacc
import concourse.bass as bass
import concourse.mybir as mybir
import concourse.tile as tile
from concourse.bass import IndirectOffsetOnAxis
from concourse.bass_utils import run_bass_kernel_spmd

F32 = mybir.dt.float32
F16 = mybir.dt.float16
I32 = mybir.dt.int32
I16 = mybir.dt.int16
AF = mybir.ActivationFunctionType
OP = mybir.AluOpType

P = 128
NCC = 8
PHASES = int(os.environ.get("GAT_PHASES", "4"))
GK = int(os.environ.get("GAT_GK", "1"))     # indirect-gather column batch
NEG_SLOPE = 0.2
F16_INPUTS = {"htiles", "W1ext", "W2ext", "alrep", "al2rep", "ident16",
              "maskt"}


@dataclass
class Prob:
    N: int
    IN_DIM: int
    H1: int
    HID: int
    OUT_DIM: int
    rounds: int = 0
    DD: list = field(default_factory=list)
    DA: list = field(default_factory=list)
    DB: list = field(default_factory=list)
    C1: int = 0
    NT: int = 0
    SD: int = 0

    def finish(self):
        self.C1 = self.H1 * self.HID
        self.NT = self.rounds * P
        self.SD = int(sum(self.DD))
        return self


def prep_all(inputs, pr: Prob):
    src = np.asarray(inputs["src"]).astype(np.int64)
    dst = np.asarray(inputs["dst"]).astype(np.int64)
    h = np.asarray(inputs["h"], dtype=np.float32)
    W1 = np.asarray(inputs["W1"], dtype=np.float32)
    al1 = np.asarray(inputs["al1"], dtype=np.float32)
    ar1 = np.asarray(inputs["ar1"], dtype=np.float32)
    b1 = np.asarray(inputs["b1"], dtype=np.float32)
    W2 = np.asarray(inputs["W2"], dtype=np.float32)
    al2 = np.asarray(inputs["al2"], dtype=np.float32)
    ar2 = np.asarray(inputs["ar2"], dtype=np.float32)
    b2 = np.asarray(inputs["b2"], dtype=np.float32)
    N = pr.N

    selfmask = src == dst
    m_cnt = np.bincount(dst[selfmask], minlength=N)      # self multiplicity
    ns_src = src[~selfmask]
    ns_dst = dst[~selfmask]
    deg = np.bincount(ns_dst, minlength=N)               # non-self in-degree

    NT_G = ((N + NCC * P - 1) // (NCC * P)) * (NCC * P)
    rounds = NT_G // (NCC * P)
    n_dummy = NT_G - N
    HALFT = NT_G // 2        # rows per table half
    halfNT = NT_G // (2 * NCC)   # rows per core per half

    # greedy A/B designation balancing each dst's in-neighbor split
    o2 = np.argsort(ns_src, kind="stable")
    odst = ns_dst[o2]
    ost = np.zeros(N + 1, np.int64)
    np.cumsum(np.bincount(ns_src, minlength=N), out=ost[1:])
    imb = np.zeros(N, np.int64)
    desA = np.zeros(N, bool)
    cntA = 0
    for v in np.random.default_rng(2).permutation(N):
        nbrs = odst[ost[v]:ost[v + 1]]
        s = np.sign(imb[nbrs]).sum()
        if (s < 0 or (s == 0 and cntA < N // 2)) and cntA < N // 2:
            desA[v] = True
            cntA += 1
            imb[nbrs] += 1
        else:
            imb[nbrs] -= 1
    a_cnt = np.bincount(ns_dst[desA[ns_src]], minlength=N)
    b_cnt = deg - a_cnt

    # pools: A-designated real nodes lex-sorted by (a, b), padded with
    # dummies (-1) to HALFT; B likewise
    selA = np.nonzero(desA)[0]
    selB = np.nonzero(~desA)[0]
    pa = selA[np.lexsort((b_cnt[selA], a_cnt[selA]))]
    pb = selB[np.lexsort((b_cnt[selB], a_cnt[selB]))]
    pa = np.concatenate([pa, np.full(HALFT - len(pa), -1, np.int64)])
    pb = np.concatenate([pb, np.full(HALFT - len(pb), -1, np.int64)])

    HW = NCC * 64   # pool window per round (512)
    DA = np.zeros(rounds, np.int64)
    DB = np.zeros(rounds, np.int64)
    for r in range(rounds):
        w = np.concatenate([pa[r * HW:(r + 1) * HW], pb[r * HW:(r + 1) * HW]])
        w = w[w >= 0]
        DA[r] = max(1, int(a_cnt[w].max()) if len(w) else 1)
        DB[r] = max(1, int(b_cnt[w].max()) if len(w) else 1)

    pr.rounds = rounds
    pr.DD = [int(DA[i] + DB[i]) for i in range(rounds)]
    pr.DA = [int(x) for x in DA]
    pr.DB = [int(x) for x in DB]
    pr.finish()
    offs = np.zeros(rounds + 1, np.int64)
    np.cumsum(np.array(pr.DD), out=offs[1:])

    # core_nodes: partitions 0:64 = A-pool block, 64:128 = B-pool block
    core_nodes = np.zeros((NCC, pr.NT), np.int64)
    for r in range(rounds):
        for c in range(NCC):
            blk = (c + r) % NCC
            core_nodes[c, r * P:r * P + 64] = \
                pa[r * HW + blk * 64:r * HW + (blk + 1) * 64]
            core_nodes[c, r * P + 64:(r + 1) * P] = \
                pb[r * HW + blk * 64:r * HW + (blk + 1) * 64]

    # node -> row within its half (A half and B half each HALFT rows)
    posh = np.full(N, -1, np.int64)
    for c in range(NCC):
        nn = core_nodes[c]
        for r in range(rounds):
            za = nn[r * P:r * P + 64]
            zb = nn[r * P + 64:(r + 1) * P]
            va = za >= 0
            vb = zb >= 0
            posh[za[va]] = c * halfNT + r * 64 + np.nonzero(va)[0]
            posh[zb[vb]] = c * halfNT + r * 64 + np.nonzero(vb)[0]

    # CSR over non-self edges by dst
    sort = np.argsort(ns_dst, kind="stable")
    s_src = ns_src[sort]
    starts = np.zeros(N + 1, np.int64)
    np.cumsum(deg, out=starts[1:])

    def wrap16(flat_idx):
        n = len(flat_idx)
        S = max(1, (n + 15) // 16)
        t = np.zeros((16, S), np.int16)
        ii = np.arange(n)
        t[ii % 16, ii // 16] = flat_idx
        return np.tile(t, (8, 1))

    per_core = []
    for c in range(NCC):
        nodes = core_nodes[c]
        mask = np.full((P, pr.SD), np.float16(-60000.0), np.float16)
        mrow = np.ones((P, rounds), np.float32)
        gi = []
        for r in range(rounds):
            da, db = pr.DA[r], pr.DB[r]
            o = offs[r]
            iA = np.zeros((da, P), np.int64)
            iB = np.zeros((db, P), np.int64)
            for p in range(P):
                n = nodes[r * P + p]
                if n < 0:
                    continue
                mrow[p, r] = max(1, int(m_cnt[n]))
                ss = s_src[starts[n]:starts[n + 1]]
                sa = ss[desA[ss]]
                sb = ss[~desA[ss]]
                iA[0:len(sa), p] = posh[sa]
                iB[0:len(sb), p] = posh[sb]
                mask[p, o:o + len(sa)] = 0.0
                mask[p, o + da:o + da + len(sb)] = 0.0
            gi.append(iA.reshape(-1))
            gi.append(iB.reshape(-1))
        gidx = wrap16(np.concatenate(gi)).astype(np.int16)
        per_core.append(dict(gidx=gidx, maskt=mask, mrow=mrow))

    # h tiles per core: own nodes' h rows, fp16, transposed for matmul lhsT
    # layout [P(k within chunk), rounds*2(chunk-major per round), P(node)]
    h_own = np.zeros((NCC, pr.NT, pr.IN_DIM), np.float32)
    for c in range(NCC):
        nn = core_nodes[c]
        valid = nn >= 0
        h_own[c, valid] = h[nn[valid]]
    ht = h_own.reshape(NCC, rounds, P, 2, P).transpose(0, 4, 1, 3, 2)
    ht = np.ascontiguousarray(ht.reshape(NCC, P, rounds * 2, P)
                              .astype(np.float16))

    # extended weights (f64 host precompute)
    H1n, HID = pr.H1, pr.HID
    al_bd = np.zeros((pr.C1, H1n), np.float64)
    ar_bd = np.zeros((pr.C1, H1n), np.float64)
    for hh in range(H1n):
        al_bd[hh * HID:(hh + 1) * HID, hh] = al1[hh].astype(np.float64)
        ar_bd[hh * HID:(hh + 1) * HID, hh] = ar1[hh].astype(np.float64)
    W1f = W1.astype(np.float64)
    W1ext = np.concatenate([W1, (W1f @ al_bd).astype(np.float32),
                            (W1f @ ar_bd).astype(np.float32)], axis=1)
    W2f = W2.astype(np.float64)
    W2ext = np.concatenate(
        [W2, (W2f @ al2.astype(np.float64).reshape(-1, 1)).astype(np.float32),
         (W2f @ ar2.astype(np.float64).reshape(-1, 1)).astype(np.float32)],
        axis=1)

    shared = dict(
        W1ext=np.ascontiguousarray(W1ext.astype(np.float16)),
        W2ext=np.ascontiguousarray(W2ext.astype(np.float16)),
        alrep=np.broadcast_to(al1.reshape(1, pr.C1).astype(np.float16),
                              (P, pr.C1)).copy(),
        al2rep=np.broadcast_to(al2.reshape(1, pr.OUT_DIM).astype(np.float16),
                               (P, pr.OUT_DIM)).copy(),
        ident16=np.eye(P, dtype=np.float16),
        b1rep=np.broadcast_to(b1, (P, pr.C1)).copy(),
        b2rep=np.broadcast_to(b2, (P, pr.OUT_DIM)).copy(),
    )
    in_maps = []
    for c in range(NCC):
        mm = dict(shared)
        mm.update(per_core[c])
        mm["htiles"] = ht[c]
        in_maps.append(mm)
    sched = dict(core_nodes=core_nodes, rounds=rounds)
    return sched, in_maps


def build_kernel_fn(pr: Prob):
    rounds, DD = pr.rounds, pr.DD
    DA, DB = pr.DA, pr.DB
    C1, H1, HID, OUT = pr.C1, pr.H1, pr.HID, pr.OUT_DIM
    NT = pr.NT
    halfNT = NT // 2
    HALFT = NCC * halfNT
    Z1W = C1 // 2 + 64     # row: feat fp16 (512B) + el f32 (32B) + pad
    Z2W = OUT             # fp16 feat2 (128B) + pad -> 256B rows for dma_gather
    offs = np.zeros(rounds + 1, np.int64)
    np.cumsum(np.array(DD), out=offs[1:])
    gi_off = [0]
    for r in range(rounds):
        gi_off.append(gi_off[-1] + (DA[r] + DB[r]) * 8)

    dbg = int(os.environ.get("GAT_DEBUG", "0"))

    def kern(tc: tile.TileContext, outs, ins):
        nc = tc.nc

        z1shard = nc.dram_tensor("z1shardd", [NT, Z1W], F32)
        z2shard = nc.dram_tensor("z2shardd", [NT, Z2W], F32)
        Z1 = nc.dram_tensor("Z1d", [NCC * NT, Z1W], F32, addr_space="Shared")
        Z2 = nc.dram_tensor("Z2d", [NCC * NT, Z2W], F32, addr_space="Shared")
        if dbg:
            d1 = nc.dram_tensor("dbg1", [NCC * NT, Z1W], F32,
                                kind="ExternalOutput")
            d2 = nc.dram_tensor("dbg2", [NCC * NT, Z2W], F32,
                                kind="ExternalOutput")
        if dbg >= 2:
            dg = nc.dram_tensor("dbgg", [P, DD[0], Z1W], F32,
                                kind="ExternalOutput")
            dew = nc.dram_tensor("dbgew", [P, DD[0] + 1, H1], F32,
                                 kind="ExternalOutput")
            dh1 = nc.dram_tensor("dbgh1", [P, rounds, C1], F32,
                                 kind="ExternalOutput")

        with (
            tc.tile_pool(name="const", bufs=1) as cpool,
            tc.tile_pool(name="big", bufs=1) as big,
        ):
            # ---- constants ----
            w1e = cpool.tile([P, 2, C1 + 2 * H1], F16)
            for c in range(2):
                nc.sync.dma_start(w1e[:, c, :], ins["W1ext"][c * P:(c + 1) * P, :])
            w2e = cpool.tile([P, 2, OUT + 2], F16)
            for c in range(2):
                nc.sync.dma_start(w2e[:, c, :], ins["W2ext"][c * P:(c + 1) * P, :])
            ident16 = cpool.tile([P, P], F16)
            nc.sync.dma_start(ident16[:], ins["ident16"][:, :])
            alrep = cpool.tile([P, C1], F16)
            nc.sync.dma_start(alrep[:], ins["alrep"][:, :])
            al2rep = cpool.tile([P, OUT], F16)
            nc.sync.dma_start(al2rep[:], ins["al2rep"][:, :])
            b1r = cpool.tile([P, C1], F32)
            nc.sync.dma_start(b1r[:], ins["b1rep"][:, :])
            b2r = cpool.tile([P, OUT], F32)
            nc.sync.dma_start(b2r[:], ins["b2rep"][:, :])
            gidx = cpool.tile([P, gi_off[-1]], I16)
            nc.sync.dma_start(gidx[:], ins["gidx"][:, :])
            maskt = cpool.tile([P, pr.SD], F16)
            nc.sync.dma_start(maskt[:], ins["maskt"][:, :])
            mrow = cpool.tile([P, rounds], F32)
            nc.sync.dma_start(mrow[:], ins["mrow"][:, :])

            feat_own = big.tile([P, rounds, C1], F16)
            el_own = big.tile([P, rounds, H1], F32)
            er_own = big.tile([P, rounds, H1], F32)
            feat2_own = big.tile([P, rounds, OUT], F16)
            eler2_own = big.tile([P, rounds, 2], F32)

            if PHASES < 1:
                return
            # ---- phase 0: z1 shard = h_own @ [W1|W1al|W1ar] (fp16) ----
            with (
                nc.named_scope("p0"),
                tc.tile_pool(name="p0h", bufs=3) as p0h,
                tc.tile_pool(name="p0ps", bufs=3, space="PSUM") as p0ps,
                tc.tile_pool(name="p0z", bufs=3) as p0z,
            ):
                for r in range(rounds):
                    htl = p0h.tile([P, 2, P], F16, tag="ht")
                    nc.sync.dma_start(htl[:], ins["htiles"][:, 2 * r:2 * r + 2, :])
                    zps = p0ps.tile([P, C1 + 2 * H1], F32)
                    for c in range(2):
                        nc.tensor.matmul(zps[:], lhsT=htl[:, c, :],
                                         rhs=w1e[:, c, :], start=(c == 0),
                                         stop=(c == 1))
                    zsb = p0z.tile([P, Z1W], F32, tag="zsb")
                    nc.vector.tensor_copy(zsb[:, 0:C1 // 2].bitcast(F16),
                                          zps[:, 0:C1])
                    nc.vector.tensor_copy(zsb[:, C1 // 2:C1 // 2 + H1],
                                          zps[:, C1:C1 + H1])
                    nc.vector.tensor_copy(el_own[:, r, :], zps[:, C1:C1 + H1])
                    nc.vector.tensor_copy(er_own[:, r, :],
                                          zps[:, C1 + H1:C1 + 2 * H1])
                    nc.vector.tensor_copy(feat_own[:, r, :],
                                          zsb[:, 0:C1 // 2].bitcast(F16))
                    nc.sync.dma_start(z1shard[r * 64:(r + 1) * 64, :],
                                      zsb[0:64, :])
                    nc.sync.dma_start(
                        z1shard[halfNT + r * 64:halfNT + (r + 1) * 64, :],
                        zsb[64:P, :])

            with nc.named_scope("ag1"):
                nc.gpsimd.collective_compute(
                    "AllGather", OP.bypass, replica_groups=[list(range(NCC))],
                    ins=[z1shard[0:halfNT, :]], outs=[Z1[0:HALFT, :]])
                nc.gpsimd.collective_compute(
                    "AllGather", OP.bypass, replica_groups=[list(range(NCC))],
                    ins=[z1shard[halfNT:NT, :]], outs=[Z1[HALFT:2 * HALFT, :]])

            if PHASES < 2:
                return
            # ---- layer-1 edge phase (+ fused layer-2 matmul) ----
            with (
                nc.named_scope("l1edge"),
                tc.tile_pool(name="fg", bufs=3) as fgp,
                tc.tile_pool(name="ew", bufs=3) as ewp,
                tc.tile_pool(name="msg", bufs=2) as msgp,
                tc.tile_pool(name="l1ps", bufs=2, space="PSUM") as l1ps,
                tc.tile_pool(name="ep", bufs=3) as epp,
                tc.tile_pool(name="tps", bufs=2, space="PSUM") as tpsp,
                tc.tile_pool(name="h1t", bufs=2) as h1tp,
                tc.tile_pool(name="z2ps", bufs=2, space="PSUM") as z2psp,
                tc.tile_pool(name="z2s", bufs=3) as z2sp,
            ):
                for r in range(rounds):
                    dd = DD[r]
                    da, db = DA[r], DB[r]
                    o = int(offs[r])
                    c0 = gi_off[r]
                    g = fgp.tile([P, dd, Z1W], F32, tag="g")
                    nc.gpsimd.dma_gather(
                        g[:, 0:da, :], Z1[0:HALFT, :],
                        gidx[:, c0:c0 + da * 8], da * P, da * P, Z1W,
                        single_packet=False)
                    nc.gpsimd.dma_gather(
                        g[:, da:dd, :], Z1[HALFT:2 * HALFT, :],
                        gidx[:, c0 + da * 8:c0 + dd * 8], db * P, db * P, Z1W,
                        single_packet=False)
                    # el rides in the gathered rows (f32 words after feat)
                    mg = msgp.tile([P, dd + 1, C1], F16, tag="mg")
                    ew = ewp.tile([P, dd + 1, H1], F32, tag="ew")
                    nc.vector.tensor_copy(ew[:, 0:dd, :],
                                          g[:, :, C1 // 2:C1 // 2 + H1])
                    nc.vector.tensor_copy(ew[:, dd, :], el_own[:, r, :])
                    nc.vector.tensor_tensor(
                        out=ew[:], in0=ew[:],
                        in1=er_own[:, r, None, :].to_broadcast((P, dd + 1, H1)),
                        op=OP.add)
                    nc.vector.tensor_tensor(
                        out=ew[:, 0:dd, :], in0=ew[:, 0:dd, :],
                        in1=maskt[:, o:o + dd, None].to_broadcast((P, dd, H1)),
                        op=OP.add)
                    lr = ewp.tile([P, dd + 1, H1], F32, tag="lr")
                    nc.vector.tensor_scalar_mul(lr[:], ew[:], NEG_SLOPE)
                    nc.vector.tensor_tensor(out=ew[:], in0=ew[:], in1=lr[:],
                                            op=OP.max)
                    nc.scalar.activation(out=ew[:], in_=ew[:], func=AF.Exp)
                    nc.vector.tensor_tensor(
                        out=ew[:, dd, :], in0=ew[:, dd, :],
                        in1=mrow[:, r:r + 1].to_broadcast((P, H1)), op=OP.mult)
                    den = ewp.tile([P, H1], F32, tag="den")
                    nc.vector.reduce_sum(
                        out=den[:], in_=ew[:].rearrange("p d h -> p h d"),
                        axis=mybir.AxisListType.X)
                    nc.vector.reciprocal(out=den[:], in_=den[:])
                    # messages (fp16) + identity-matmul accumulation
                    nc.vector.tensor_tensor(
                        out=mg[:, 0:dd, :].rearrange("p d (h w) -> p d h w",
                                                     h=H1),
                        in0=g[:, :, 0:C1 // 2].bitcast(F16).rearrange(
                            "p d (h w) -> p d h w", h=H1),
                        in1=ew[:, 0:dd, :, None].to_broadcast((P, dd, H1, HID)),
                        op=OP.mult)
                    nc.vector.tensor_tensor(
                        out=mg[:, dd, :].rearrange("p (h w) -> p h w", h=H1),
                        in0=feat_own[:, r, :].rearrange("p (h w) -> p h w", h=H1),
                        in1=ew[:, dd, :, None].to_broadcast((P, H1, HID)),
                        op=OP.mult)
                    ps = l1ps.tile([P, C1], F32)
                    for k in range(dd + 1):
                        nc.tensor.matmul(ps[:], lhsT=ident16[:], rhs=mg[:, k, :],
                                         start=(k == 0), stop=(k == dd))
                    # h1 = elu(ps/den + b1)
                    x = epp.tile([P, C1], F32, tag="x")
                    nc.vector.tensor_tensor(
                        out=x[:].rearrange("p (h w) -> p h w", h=H1),
                        in0=ps[:].rearrange("p (h w) -> p h w", h=H1),
                        in1=den[:, :, None].to_broadcast((P, H1, HID)),
                        op=OP.mult)
                    nc.vector.tensor_tensor(out=x[:], in0=x[:], in1=b1r[:],
                                            op=OP.add)
                    mn = epp.tile([P, C1], F32, tag="mn")
                    nc.vector.tensor_scalar_min(mn[:], x[:], 0.0)
                    nc.scalar.activation(out=mn[:], in_=mn[:], func=AF.Exp)
                    nc.vector.tensor_scalar_max(x[:], x[:], 0.0)
                    nc.vector.tensor_tensor(out=x[:], in0=x[:], in1=mn[:],
                                            op=OP.add)
                    h1r = epp.tile([P, C1], F16, tag="h1r")
                    nc.vector.tensor_scalar_sub(h1r[:], x[:], 1.0)
                    if dbg >= 2:
                        nc.sync.dma_start(dh1[:, r, :], x[:])
                        if r == 0:
                            nc.sync.dma_start(dg[:, :, :], g[:])
                            nc.sync.dma_start(dew[:, :, :], ew[:])
                    # fused layer-2 matmul for this round
                    tps = tpsp.tile([P, 2, P], F16)
                    for c in range(2):
                        nc.tensor.transpose(out=tps[:, c, :],
                                            in_=h1r[:, c * P:(c + 1) * P],
                                            identity=ident16[:])
                    h1t = h1tp.tile([P, 2, P], F16, tag="h1t")
                    nc.vector.tensor_copy(h1t[:], tps[:])
                    z2ps = z2psp.tile([P, OUT + 2], F32)
                    for c in range(2):
                        nc.tensor.matmul(z2ps[:], lhsT=h1t[:, c, :],
                                         rhs=w2e[:, c, :],
                                         start=(c == 0), stop=(c == 1))
                    zsb2 = z2sp.tile([P, Z2W], F32, tag="zsb2")
                    nc.vector.tensor_copy(zsb2[:, 0:OUT // 2].bitcast(F16),
                                          z2ps[:, 0:OUT])
                    nc.vector.tensor_copy(eler2_own[:, r, :],
                                          z2ps[:, OUT:OUT + 2])
                    nc.vector.tensor_copy(feat2_own[:, r, :],
                                          zsb2[:, 0:OUT // 2].bitcast(F16))
                    nc.sync.dma_start(z2shard[r * 64:(r + 1) * 64, :],
                                      zsb2[0:64, :])
                    nc.sync.dma_start(
                        z2shard[halfNT + r * 64:halfNT + (r + 1) * 64, :],
                        zsb2[64:P, :])

            with nc.named_scope("ag2"):
                nc.gpsimd.collective_compute(
                    "AllGather", OP.bypass, replica_groups=[list(range(NCC))],
                    ins=[z2shard[0:halfNT, :]], outs=[Z2[0:HALFT, :]])
                nc.gpsimd.collective_compute(
                    "AllGather", OP.bypass, replica_groups=[list(range(NCC))],
                    ins=[z2shard[halfNT:NT, :]], outs=[Z2[HALFT:2 * HALFT, :]])
            if dbg:
                nc.sync.dma_start(d1[:, :], Z1[:, :])
                nc.sync.dma_start(d2[:, :], Z2[:, :])

            if PHASES < 3:
                return
            # ---- layer-2 edge phase ----
            with (
                nc.named_scope("l2edge"),
                tc.tile_pool(name="fg2", bufs=3) as fg2p,
                tc.tile_pool(name="ew2", bufs=3) as ew2p,
                tc.tile_pool(name="msg2", bufs=3) as msg2p,
                tc.tile_pool(name="l2ps", bufs=2, space="PSUM") as l2ps,
            ):
                for r in range(rounds):
                    dd = DD[r]
                    da, db = DA[r], DB[r]
                    o = int(offs[r])
                    c0 = gi_off[r]
                    g2 = fg2p.tile([P, dd, Z2W], F32, tag="g2")
                    nc.gpsimd.dma_gather(
                        g2[:, 0:da, :], Z2[0:HALFT, :],
                        gidx[:, c0:c0 + da * 8], da * P, da * P, Z2W,
                        single_packet=False)
                    nc.gpsimd.dma_gather(
                        g2[:, da:dd, :], Z2[HALFT:2 * HALFT, :],
                        gidx[:, c0 + da * 8:c0 + dd * 8], db * P, db * P, Z2W,
                        single_packet=False)
                    mg = msg2p.tile([P, dd + 1, OUT], F16, tag="mg2")
                    nc.vector.tensor_tensor(
                        out=mg[:, 0:dd, :],
                        in0=g2[:, :, 0:OUT // 2].bitcast(F16),
                        in1=al2rep[:, None, :].to_broadcast((P, dd, OUT)),
                        op=OP.mult)
                    ew = ew2p.tile([P, dd + 1], F32, tag="ew2")
                    nc.vector.reduce_sum(out=ew[:, 0:dd], in_=mg[:, 0:dd, :],
                                         axis=mybir.AxisListType.X)
                    nc.vector.tensor_copy(ew[:, dd:dd + 1],
                                          eler2_own[:, r, 0:1])
                    nc.vector.tensor_tensor(
                        out=ew[:], in0=ew[:],
                        in1=eler2_own[:, r, 1:2].to_broadcast((P, dd + 1)),
                        op=OP.add)
                    nc.vector.tensor_tensor(
                        out=ew[:, 0:dd], in0=ew[:, 0:dd],
                        in1=maskt[:, o:o + dd], op=OP.add)
                    lr2 = ew2p.tile([P, dd + 1], F32, tag="lr2")
                    nc.vector.tensor_scalar_mul(lr2[:], ew[:], NEG_SLOPE)
                    nc.vector.tensor_tensor(out=ew[:], in0=ew[:], in1=lr2[:],
                                            op=OP.max)
                    nc.scalar.activation(out=ew[:], in_=ew[:], func=AF.Exp)
                    nc.vector.tensor_tensor(
                        out=ew[:, dd:dd + 1], in0=ew[:, dd:dd + 1],
                        in1=mrow[:, r:r + 1], op=OP.mult)
                    den = ew2p.tile([P, 1], F32, tag="den2")
                    nc.vector.reduce_sum(out=den[:], in_=ew[:],
                                         axis=mybir.AxisListType.X)
                    nc.vector.reciprocal(out=den[:], in_=den[:])
                    nc.vector.tensor_tensor(
                        out=mg[:, 0:dd, :],
                        in0=g2[:, :, 0:OUT // 2].bitcast(F16),
                        in1=ew[:, 0:dd, None].to_broadcast((P, dd, OUT)),
                        op=OP.mult)
                    nc.vector.tensor_tensor(
                        out=mg[:, dd, :], in0=feat2_own[:, r, :],
                        in1=ew[:, dd:dd + 1].to_broadcast((P, OUT)),
                        op=OP.mult)
                    ps = l2ps.tile([P, OUT], F32)
                    for k in range(dd + 1):
                        nc.tensor.matmul(ps[:], lhsT=ident16[:], rhs=mg[:, k, :],
                                         start=(k == 0), stop=(k == dd))
                    ot = ew2p.tile([P, OUT], F32, tag="ot")
                    nc.vector.tensor_tensor(
                        out=ot[:], in0=ps[:],
                        in1=den[:].to_broadcast((P, OUT)), op=OP.mult)
                    nc.vector.tensor_tensor(out=ot[:], in0=ot[:], in1=b2r[:],
                                            op=OP.add)
                    nc.sync.dma_start(outs["out"][:, r, :], ot[:])

    return kern


def declare_io(nc, in_maps, pr: Prob):
    ins_ap = {}
    for k, v in in_maps[0].items():
        if k in F16_INPUTS:
            dt = F16
        else:
            dt = mybir.dt.from_np(v.dtype)
        ins_ap[k] = nc.dram_tensor(
            f"in_{k}", list(v.shape), dt, kind="ExternalInput").ap()
    outs_ap = {"out": nc.dram_tensor(
        "out", [P, pr.rounds, pr.OUT_DIM], F32, kind="ExternalOutput").ap()}
    return ins_ap, outs_ap


def assemble_output(results, sched, pr: Prob):
    out = np.zeros((pr.N, pr.OUT_DIM), np.float32)
    for c in range(NCC):
        nodes = sched["core_nodes"][c]
        oc = results[c]["out"]  # [P, rounds, OUT]
        oc = oc.transpose(1, 0, 2).reshape(pr.NT, pr.OUT_DIM)
        valid = nodes >= 0
        out[nodes[valid]] = oc[valid]
    return out


def _build_and_run(inputs, trace=False):
    pr = Prob(N=50000, IN_DIM=256, H1=8, HID=32, OUT_DIM=64)
    sched, in_maps = prep_all(inputs, pr)
    nc = bacc.Bacc("TRN2", target_bir_lowering=False, debug=False,
                   num_devices=NCC)
    ins_ap, outs_ap = declare_io(nc, in_maps, pr)
    kern = build_kernel_fn(pr)
    with tile.TileContext(nc) as tc:
        kern(tc, outs_ap, ins_ap)
    nc.compile()
    maps = [{f"in_{k}": v for k, v in m.items()} for m in in_maps]
    res = run_bass_kernel_spmd(nc, maps, core_ids=list(range(NCC)),
                               trace=trace)
    return res, sched, pr


def kernel(**inputs) -> np.ndarray:
    res, sched, pr = _build_and_run(inputs, trace=False)
    return assemble_output(res.results, sched, pr)


def kernel_timed(inputs):
    import time
    trace = bool(int(os.environ.get("GAT_TRACE", "0")))
    if trace:
        import trace_hook
        trace_hook.install()
    t0 = time.perf_counter()
    res, sched, pr = _build_and_run(inputs, trace=trace)
    t1 = time.perf_counter()
    print(f"build+run {t1-t0:.1f}s")
    if trace:
        print(f"exec_time_ns: {res.exec_time_ns}")
        if res.per_core_scope_times:
            for scope, d in sorted(res.per_core_scope_times.items()):
                print(f"  scope {scope}: "
                      + " ".join(f"c{c}={v}" for c, v in sorted(d.items())))
        if res.instructions_and_trace:
            print("trace path:", res.instructions_and_trace[1])
    return assemble_output(res.results, sched, pr)


if __name__ == "__main__":
    import pickle
    with open("/tmp/inputs.pkl", "rb") as f:
        inputs = pickle.load(f)
    out = kernel_timed(inputs)
    exp = np.load("/tmp/expected_np.npy")
    rel = np.linalg.norm(out - exp) / np.linalg.norm(exp)
    print("Relative error:", rel)
